# revision 6
# baseline (speedup 1.0000x reference)
"""CrossAttentionFusion kernel for 8 Trainium2 NeuronCores.

Math (per reference): two seq-len-1 cross-attention blocks (each reduces to
out_proj(v_proj(x)) = one fused E x E matmul), residual+LN after each, then a
4E FFN with exact-erf GELU and a final residual+LN.

Strategy:
  - Pure data parallel over the batch (16384 rows -> 2048 rows per core).
  - Feature-major ("transposed") activations on device: tiles are
    [128 features, batch] so every matmul is lhsT(=W.T chunk).T @ x.T with no
    on-device transposes. LayerNorm reductions over features run on the PE
    (ones-vector matmuls) with K=1 broadcast matmuls for mean/rstd.
  - f32r (TF32-like fast fp32) matmuls: 4x the plain-fp32 rate.
  - Attention pairs are fused on the host: W1 = w_out1 @ wv1, b1 = w_out1 @
    bv1 + b_out1 (exact algebra, seq_len==1).
  - FFN hidden h [4096 x batch] is spilled through DRAM between ffn1/ffn2.
"""

import os
import sys

import numpy as np

sys.path.insert(0, "/opt/trn_rl_repo")

E = 1024
B = 16384
NCORES = 8
R = B // NCORES          # rows per core
CH = E // 128            # feature chunks (8)
F = 4 * E                # ffn hidden (4096)
FCH = F // 128           # ffn hidden chunks (32)
NGRP = 4                 # ffn1 weight pieces (each 1024 wide)
N = 512                  # batch tile for phases AB/C
NT = R // N              # 4
N2 = 256                 # batch tile for phase D
NH = R // N2             # 8

# CoreSim does not implement Gelu; tests may set KERNEL_GELU=Tanh for
# structural sim checks. Hardware always uses the real (erf) Gelu.
_GELU_FUNC = os.environ.get("KERNEL_GELU", "Gelu")

_RUNNER = None


def _emit_program(nc, repeats=1):
    import concourse.bass as bass
    import concourse.mybir as mybir
    import concourse.tile as tile

    F32 = mybir.dt.float32
    F32R = mybir.dt.float32r
    AF = mybir.ActivationFunctionType
    OP = mybir.AluOpType
    ts = bass.ts

    xt = nc.declare_dram_parameter("xt", [E, R], F32R, isOutput=False)
    it = nc.declare_dram_parameter("it", [E, R], F32R, isOutput=False)
    w1t = nc.declare_dram_parameter("w1t", [E, E], F32R, isOutput=False)
    w2t = nc.declare_dram_parameter("w2t", [E, E], F32R, isOutput=False)
    wf1t = nc.declare_dram_parameter("wf1t", [E, F], F32R, isOutput=False)
    wf2t = nc.declare_dram_parameter("wf2t", [F, E], F32R, isOutput=False)
    # packed per-partition params: [128, c] with [p, c] = v[c*128+p]
    bias1 = nc.declare_dram_parameter("bias1", [128, CH], F32, isOutput=False)
    bias2 = nc.declare_dram_parameter("bias2", [128, CH], F32, isOutput=False)
    bf1 = nc.declare_dram_parameter("bf1", [128, FCH], F32, isOutput=False)
    bf2 = nc.declare_dram_parameter("bf2", [128, CH], F32, isOutput=False)
    # ln params: 6 groups of CH cols: g1 be1 g2 be2 g3 be3
    lnp = nc.declare_dram_parameter("lnp", [128, 6 * CH], F32, isOutput=False)
    ones_in = nc.declare_dram_parameter("ones_in", [128, 1], F32R, isOutput=False)
    ones1_in = nc.declare_dram_parameter("ones1_in", [1, 128], F32R, isOutput=False)
    # f32r and f32 are bit-identical; declaring the output f32r lets the LN3
    # result DMA straight out without a cast.
    ot = nc.declare_dram_parameter("ot", [E, R], F32R, isOutput=True)

    xtr = xt.rearrange("(c p) r -> p c r", p=128)
    itr = it.rearrange("(c p) r -> p c r", p=128)
    otr = ot.rearrange("(c p) r -> p c r", p=128)
    w1r = w1t.rearrange("(c p) m -> p c m", p=128)
    w2r = w2t.rearrange("(c p) m -> p c m", p=128)
    wf1r = wf1t.rearrange("(k p) (g j) -> g p k j", p=128, g=NGRP)
    wf2r = wf2t.rearrange("(k p) m -> p k m", p=128)

    with nc.allow_low_precision("f32r matmul pipeline; accumulation is f32 psum"), \
         tile.TileContext(nc) as tc:
        from contextlib import ExitStack

        with tc.tile_pool(name="dram", bufs=1, space="DRAM") as dram, \
             tc.tile_pool(name="const", bufs=1) as const:
            hbuf = dram.tile([128, FCH, R], F32R)
            cbuf = dram.tile([128, CH, R], F32R)

            b1sb = const.tile([128, CH], F32)
            b2sb = const.tile([128, CH], F32)
            bf1sb = const.tile([128, FCH], F32)
            bf2sb = const.tile([128, CH], F32)
            lnsb = const.tile([128, 6 * CH], F32)
            ones128 = const.tile([128, 1], F32R)
            ones1 = const.tile([1, 128], F32R)
            epsb = const.tile([1, 1], F32)
            nc.gpsimd.dma_start(out=b1sb[:], in_=bias1[:])
            nc.gpsimd.dma_start(out=b2sb[:], in_=bias2[:])
            nc.gpsimd.dma_start(out=bf1sb[:], in_=bf1[:])
            nc.gpsimd.dma_start(out=bf2sb[:], in_=bf2[:])
            nc.gpsimd.dma_start(out=lnsb[:], in_=lnp[:])
            nc.gpsimd.dma_start(out=ones128[:], in_=ones_in[:])
            nc.gpsimd.dma_start(out=ones1[:], in_=ones1_in[:])
            nc.vector.memset(epsb[:], 1e-5)

            def layer_norm(ctx_pools, r_t, width, ln_idx, out_t):
                """LN over features of r_t [128, CH, width] -> out_t (may alias).

                Destroys r_t. ln_idx selects g/be columns in lnsb.
                ctx_pools = (sqp, stp, ps_st, ps_bc)
                """
                sqp, stp, ps_st, ps_bc = ctx_pools
                g_col = lnsb[:, 2 * ln_idx * CH: (2 * ln_idx + 1) * CH]
                be_col = lnsb[:, (2 * ln_idx + 1) * CH: (2 * ln_idx + 2) * CH]
                s_ps = ps_st.tile([1, width], F32, tag="s_ps")
                q_ps = ps_st.tile([1, width], F32, tag="q_ps")
                for m in range(CH):
                    nc.tensor.matmul(s_ps[:], ones128[:], r_t[:, m, :],
                                     start=(m == 0), stop=(m == CH - 1))
                for m in range(CH):
                    sq = sqp.tile([128, width], F32R, tag="sq")
                    nc.vector.tensor_tensor(out=sq[:], in0=r_t[:, m, :],
                                            in1=r_t[:, m, :], op=OP.mult)
                    nc.tensor.matmul(q_ps[:], ones128[:], sq[:],
                                     start=(m == 0), stop=(m == CH - 1))
                mu_t = stp.tile([1, width], F32R, tag="mu")
                var_t = stp.tile([1, width], F32, tag="var")
                rstd_t = stp.tile([1, width], F32R, tag="rstd")
                musq = stp.tile([1, width], F32, tag="musq")
                nc.vector.tensor_scalar(out=mu_t[:], in0=s_ps[:], scalar1=1.0 / E,
                                        scalar2=None, op0=OP.mult)
                nc.vector.tensor_scalar(out=var_t[:], in0=q_ps[:], scalar1=1.0 / E,
                                        scalar2=None, op0=OP.mult)
                nc.vector.tensor_tensor(out=musq[:], in0=mu_t[:], in1=mu_t[:], op=OP.mult)
                nc.vector.tensor_tensor(out=var_t[:], in0=var_t[:], in1=musq[:], op=OP.subtract)
                nc.scalar.activation(out=var_t[:], in_=var_t[:], func=AF.Sqrt, bias=epsb[:])
                nc.vector.reciprocal(out=rstd_t[:], in_=var_t[:])
                mu_b = ps_bc.tile([128, width], F32, tag="mu_b")
                rstd_b = ps_bc.tile([128, width], F32, tag="rstd_b")
                nc.tensor.matmul(mu_b[:], ones1[:], mu_t[:], start=True, stop=True)
                nc.tensor.matmul(rstd_b[:], ones1[:], rstd_t[:], start=True, stop=True)
                for m in range(CH):
                    nc.vector.tensor_tensor(out=r_t[:, m, :], in0=r_t[:, m, :],
                                            in1=mu_b[:], op=OP.subtract)
                    nc.vector.tensor_tensor(out=r_t[:, m, :], in0=r_t[:, m, :],
                                            in1=rstd_b[:], op=OP.mult)
                    nc.vector.tensor_scalar(out=out_t[:, m, :], in0=r_t[:, m, :],
                                            scalar1=g_col[:, m:m + 1],
                                            scalar2=be_col[:, m:m + 1],
                                            op0=OP.mult, op1=OP.add)

            for rep in range(repeats):
                # ---------------- Phase AB: att1+LN1+att2+LN2 -> c ----------------
                with ExitStack() as ab:
                    wab = ab.enter_context(tc.tile_pool(name="wab", bufs=1))
                    px = ab.enter_context(tc.tile_pool(name="px", bufs=2))
                    pit = ab.enter_context(tc.tile_pool(name="pit", bufs=3))
                    pr = ab.enter_context(tc.tile_pool(name="pr", bufs=2))
                    pimg = ab.enter_context(tc.tile_pool(name="pimg", bufs=2))
                    sqp = ab.enter_context(tc.tile_pool(name="sqp", bufs=2))
                    stp = ab.enter_context(tc.tile_pool(name="stp", bufs=2))
                    psA = ab.enter_context(tc.tile_pool(name="psA", bufs=3, space="PSUM"))
                    ps_st = ab.enter_context(tc.tile_pool(name="ps_st", bufs=1, space="PSUM"))
                    ps_bc = ab.enter_context(tc.tile_pool(name="ps_bc", bufs=1, space="PSUM"))
                    lnpools = (sqp, stp, ps_st, ps_bc)

                    w1sb = wab.tile([128, CH, E], F32R)
                    w2sb = wab.tile([128, CH, E], F32R)
                    nc.sync.dma_start(out=w1sb[:], in_=w1r[:])
                    nc.sync.dma_start(out=w2sb[:], in_=w2r[:])

                    for n in range(NT):
                        sl = slice(n * N, (n + 1) * N)
                        xt_t = px.tile([128, CH, N], F32R, tag="xt_t")
                        nc.sync.dma_start(out=xt_t[:], in_=xtr[:, :, sl])

                        r1 = pr.tile([128, CH, N], F32R, tag="r")
                        for m in range(CH):
                            acc = psA.tile([128, N], F32, tag="acc")
                            for k in range(CH):
                                nc.tensor.matmul(acc[:], w1sb[:, k, ts(m, 128)],
                                                 xt_t[:, k, :],
                                                 start=(k == 0), stop=(k == CH - 1))
                            itc = pit.tile([128, N], F32R, tag="itc")
                            nc.sync.dma_start(out=itc[:], in_=itr[:, m, sl])
                            nc.vector.scalar_tensor_tensor(
                                out=r1[:, m, :], in0=acc[:], scalar=b1sb[:, m:m + 1],
                                in1=itc[:], op0=OP.add, op1=OP.add)
                        img = pimg.tile([128, CH, N], F32R, tag="img")
                        layer_norm(lnpools, r1, N, 0, img)

                        r2 = pr.tile([128, CH, N], F32R, tag="r")
                        for m in range(CH):
                            acc = psA.tile([128, N], F32, tag="acc")
                            for k in range(CH):
                                nc.tensor.matmul(acc[:], w2sb[:, k, ts(m, 128)],
                                                 img[:, k, :],
                                                 start=(k == 0), stop=(k == CH - 1))
                            nc.vector.scalar_tensor_tensor(
                                out=r2[:, m, :], in0=acc[:], scalar=b2sb[:, m:m + 1],
                                in1=xt_t[:, m, :], op0=OP.add, op1=OP.add)
                        # LN2 -> txt2 (into r2), then c = txt2 + img (into img)
                        layer_norm(lnpools, r2, N, 1, r2)
                        for m in range(CH):
                            nc.vector.tensor_tensor(out=img[:, m, :], in0=r2[:, m, :],
                                                    in1=img[:, m, :], op=OP.add)
                        nc.sync.dma_start(out=cbuf[:, :, sl], in_=img[:])

                # ---------------- Phase C: h = gelu(wf1 @ c + bf1) ----------------
                with ExitStack() as pc:
                    pcc = pc.enter_context(tc.tile_pool(name="pcc", bufs=NT))
                    pw1 = pc.enter_context(tc.tile_pool(name="pw1", bufs=2))
                    ph = pc.enter_context(tc.tile_pool(name="ph", bufs=2))
                    psC = pc.enter_context(tc.tile_pool(name="psC", bufs=3, space="PSUM"))

                    c_ts = []
                    for n in range(NT):
                        ct = pcc.tile([128, CH, N], F32R, tag="ct")
                        nc.sync.dma_start(out=ct[:], in_=cbuf[:, :, n * N:(n + 1) * N])
                        c_ts.append(ct)
                    for g in range(NGRP):
                        wg = pw1.tile([128, CH, E], F32R, tag="wg")
                        nc.sync.dma_start(out=wg[:], in_=wf1r[g])
                        for n in range(NT):
                            hst = ph.tile([128, CH, N], F32R, tag="hst")
                            for mj in range(CH):
                                acc = psC.tile([128, N], F32, tag="accC")
                                for k in range(CH):
                                    nc.tensor.matmul(acc[:], wg[:, k, ts(mj, 128)],
                                                     c_ts[n][:, k, :],
                                                     start=(k == 0), stop=(k == CH - 1))
                                nc.scalar.activation(hst[:, mj, :], acc[:],
                                                     getattr(AF, _GELU_FUNC),
                                                     bias=bf1sb[:, g * CH + mj: g * CH + mj + 1])
                            nc.sync.dma_start(
                                out=hbuf[:, g * CH:(g + 1) * CH, n * N:(n + 1) * N],
                                in_=hst[:])

                # ---------------- Phase D: ffn2 + residual + LN3 ----------------
                with ExitStack() as pd:
                    pwf2 = pd.enter_context(tc.tile_pool(name="pwf2", bufs=1))
                    phD = pd.enter_context(tc.tile_pool(name="phD", bufs=3))
                    pcD = pd.enter_context(tc.tile_pool(name="pcD", bufs=2))
                    sqpD = pd.enter_context(tc.tile_pool(name="sqpD", bufs=2))
                    stpD = pd.enter_context(tc.tile_pool(name="stpD", bufs=2))
                    psD = pd.enter_context(tc.tile_pool(name="psD", bufs=3, space="PSUM"))
                    ps_stD = pd.enter_context(tc.tile_pool(name="ps_stD", bufs=1, space="PSUM"))
                    ps_bcD = pd.enter_context(tc.tile_pool(name="ps_bcD", bufs=1, space="PSUM"))
                    lnpoolsD = (sqpD, stpD, ps_stD, ps_bcD)

                    wf2sb = pwf2.tile([128, FCH, E], F32R)
                    nc.sync.dma_start(out=wf2sb[:], in_=wf2r[:])

                    HH = FCH // 2
                    for hn in range(NH):
                        sl = slice(hn * N2, (hn + 1) * N2)
                        # h tile split into two halves for finer DMA/compute overlap
                        hts = []
                        for half in range(2):
                            ht = phD.tile([128, HH, N2], F32R, tag="ht")
                            nc.sync.dma_start(
                                out=ht[:], in_=hbuf[:, half * HH:(half + 1) * HH, sl])
                            hts.append(ht)
                        ch = pcD.tile([128, CH, N2], F32R, tag="ch")
                        nc.sync.dma_start(out=ch[:], in_=cbuf[:, :, sl])
                        for m in range(CH):
                            acc = psD.tile([128, N2], F32, tag="accD")
                            for k in range(FCH):
                                nc.tensor.matmul(acc[:], wf2sb[:, k, ts(m, 128)],
                                                 hts[k // HH][:, k % HH, :],
                                                 start=(k == 0), stop=(k == FCH - 1))
                            nc.vector.scalar_tensor_tensor(
                                out=ch[:, m, :], in0=acc[:], scalar=bf2sb[:, m:m + 1],
                                in1=ch[:, m, :], op0=OP.add, op1=OP.add)
                        # LN3 applied in place into ch, then DMA out
                        layer_norm(lnpoolsD, ch, N2, 2, ch)
                        nc.sync.dma_start(out=otr[:, :, sl], in_=ch[:])

    nc.finalize()
    return nc


def _build(repeats=1):
    from concourse import bacc

    nc = bacc.Bacc()
    return _emit_program(nc, repeats=repeats)


def _make_exec(nc, n_cores=NCORES):
    """Cached jitted SPMD executor, mirroring run_bass_via_pjrt's multi-core
    branch so repeated calls reuse the compiled NEFF."""
    import jax
    import concourse.mybir as mybir
    from concourse import bass2jax
    from jax.experimental.shard_map import shard_map
    from jax.sharding import Mesh, PartitionSpec

    bass2jax.install_neuronx_cc_hook()

    partition_name = nc.partition_id_tensor.name if nc.partition_id_tensor else None
    in_names, out_names, out_avals, zero_shapes = [], [], [], []
    for alloc in nc.m.functions[0].allocations:
        if not isinstance(alloc, mybir.MemoryLocationSet):
            continue
        name = alloc.memorylocations[0].name
        if alloc.kind == "ExternalInput":
            if name != partition_name:
                in_names.append(name)
        elif alloc.kind == "ExternalOutput":
            out_names.append(name)
            shape = tuple(alloc.tensor_shape)
            dtype = mybir.dt.np(alloc.dtype)
            out_avals.append(jax.core.ShapedArray(shape, dtype))
            zero_shapes.append((shape, dtype))
    n_params = len(in_names)
    n_outs = len(out_names)
    all_names = in_names + out_names
    if partition_name is not None:
        all_names = all_names + [partition_name]

    def _body(*args):
        operands = list(args)
        if partition_name is not None:
            operands.append(bass2jax.partition_id_tensor())
        outs = bass2jax._bass_exec_p.bind(
            *operands,
            out_avals=tuple(out_avals),
            in_names=tuple(all_names),
            out_names=tuple(out_names),
            lowering_input_output_aliases=(),
            sim_require_finite=True,
            sim_require_nnan=True,
            nc=nc,
        )
        return tuple(outs)

    devices = jax.devices()[:n_cores]
    mesh = Mesh(np.asarray(devices), ("core",))
    in_specs = (PartitionSpec("core"),) * (n_params + n_outs)
    out_specs = (PartitionSpec("core"),) * n_outs
    donate = tuple(range(n_params, n_params + n_outs))
    sharded = jax.jit(
        shard_map(_body, mesh=mesh, in_specs=in_specs, out_specs=out_specs,
                  check_rep=False),
        donate_argnums=donate, keep_unused=True)

    def run(in_maps):
        concat_in = [
            np.concatenate([np.asarray(in_maps[c][nm]) for c in range(n_cores)], axis=0)
            for nm in in_names
        ]
        concat_zeros = [
            np.zeros((n_cores * s[0],) + tuple(s[1:]), dt) for (s, dt) in zero_shapes
        ]
        out_arrs = sharded(*concat_in, *concat_zeros)
        out_arrs = [np.asarray(a) for a in out_arrs]
        return [
            {nm: out_arrs[i].reshape(n_cores, *out_avals[i].shape)[c]
             for i, nm in enumerate(out_names)}
            for c in range(n_cores)
        ]

    run.in_names = in_names
    run.out_names = out_names
    run.sharded = sharded
    run.n_cores = n_cores
    run.out_avals = out_avals
    run.zero_shapes = zero_shapes
    run.body = _body
    run.mesh = mesh
    run.in_specs = in_specs
    run.out_specs = out_specs
    run.nc = nc
    return run


def _pack_pp(v, ch):
    """bias vector [ch*128] -> per-partition [128, ch]."""
    return np.ascontiguousarray(v.reshape(ch, 128).T.astype(np.float32))


def prepare_in_maps(img_feat, txt_feat, w_in1, b_in1, w_out1, b_out1,
                    w_in2, b_in2, w_out2, b_out2,
                    g1, be1, g2, be2, g3, be3,
                    w_ffn1, b_ffn1, w_ffn2, b_ffn2):
    f32 = np.float32
    img = np.asarray(img_feat, f32)
    txt = np.asarray(txt_feat, f32)
    w_in1 = np.asarray(w_in1, f32); b_in1 = np.asarray(b_in1, f32)
    w_out1 = np.asarray(w_out1, f32); b_out1 = np.asarray(b_out1, f32)
    w_in2 = np.asarray(w_in2, f32); b_in2 = np.asarray(b_in2, f32)
    w_out2 = np.asarray(w_out2, f32); b_out2 = np.asarray(b_out2, f32)
    w_ffn1 = np.asarray(w_ffn1, f32); b_ffn1 = np.asarray(b_ffn1, f32)
    w_ffn2 = np.asarray(w_ffn2, f32); b_ffn2 = np.asarray(b_ffn2, f32)

    wv1 = w_in1[2 * E:]
    bv1 = b_in1[2 * E:]
    W1 = w_out1 @ wv1                      # att1 == txt @ W1.T + b1
    b1 = w_out1 @ bv1 + b_out1
    wv2 = w_in2[2 * E:]
    bv2 = b_in2[2 * E:]
    W2 = w_out2 @ wv2
    b2 = w_out2 @ bv2 + b_out2

    lnp = np.concatenate([
        _pack_pp(np.asarray(v, f32), CH)
        for v in (g1, be1, g2, be2, g3, be3)], axis=1)

    shared = {
        "w1t": np.ascontiguousarray(W1.T),
        "w2t": np.ascontiguousarray(W2.T),
        "wf1t": np.ascontiguousarray(w_ffn1.T),
        "wf2t": np.ascontiguousarray(w_ffn2.T),
        "bias1": _pack_pp(b1, CH),
        "bias2": _pack_pp(b2, CH),
        "bf1": _pack_pp(b_ffn1, FCH),
        "bf2": _pack_pp(b_ffn2, CH),
        "lnp": lnp,
        "ones_in": np.ones((128, 1), f32),
        "ones1_in": np.ones((1, 128), f32),
    }
    in_maps = []
    for c in range(NCORES):
        sh = slice(c * R, (c + 1) * R)
        m = dict(shared)
        m["xt"] = np.ascontiguousarray(txt[sh].T)
        m["it"] = np.ascontiguousarray(img[sh].T)
        in_maps.append(m)
    return in_maps


def get_runner():
    global _RUNNER
    if _RUNNER is None:
        nc = _build()
        _RUNNER = _make_exec(nc)
    return _RUNNER


def kernel(**inputs) -> np.ndarray:
    run = get_runner()
    in_maps = prepare_in_maps(**inputs)
    results = run(in_maps)
    out = np.empty((B, E), np.float32)
    for c in range(NCORES):
        out[c * R:(c + 1) * R] = results[c]["ot"].T
    return out


# revision 8
# speedup vs baseline: 1.2543x; 1.2543x over previous
"""CrossAttentionFusion kernel for 8 Trainium2 NeuronCores.

Math (per reference): two seq-len-1 cross-attention blocks (each reduces to
out_proj(v_proj(x)) = one fused E x E matmul), residual+LN after each, then a
4E FFN with exact-erf GELU and a final residual+LN.

Strategy:
  - Pure data parallel over the batch (16384 rows -> 2048 rows per core).
  - Feature-major ("transposed") activations on device: tiles are
    [128 features, batch] so every matmul is lhsT(=W.T chunk).T @ x.T with no
    on-device transposes. LayerNorm reductions over features run on the PE
    (ones-vector matmuls) with K=1 broadcast matmuls for mean/rstd.
  - f32r (TF32-like fast fp32) matmuls: 4x the plain-fp32 rate.
  - Attention pairs are fused on the host: W1 = w_out1 @ wv1, b1 = w_out1 @
    bv1 + b_out1 (exact algebra, seq_len==1).
  - FFN hidden h [4096 x batch] is spilled through DRAM between ffn1/ffn2.
"""

import os
import sys

import numpy as np

sys.path.insert(0, "/opt/trn_rl_repo")

E = 1024
B = 16384
NCORES = 8
R = B // NCORES          # rows per core
CH = E // 128            # feature chunks (8)
F = 4 * E                # ffn hidden (4096)
FCH = F // 128           # ffn hidden chunks (32)
NGRP = 4                 # ffn1 weight pieces (each 1024 wide)
N = 512                  # batch tile for phases AB/C
NT = R // N              # 4
N2 = 256                 # batch tile for phase D
NH = R // N2             # 8

# CoreSim does not implement Gelu; tests may set KERNEL_GELU=Tanh for
# structural sim checks. Hardware always uses the real (erf) Gelu.
_GELU_FUNC = os.environ.get("KERNEL_GELU", "Gelu")

_RUNNER = None


def _emit_program(nc, repeats=1):
    import concourse.bass as bass
    import concourse.mybir as mybir
    import concourse.tile as tile

    F32 = mybir.dt.float32
    F32R = mybir.dt.float32r
    AF = mybir.ActivationFunctionType
    OP = mybir.AluOpType
    ts = bass.ts

    xt = nc.declare_dram_parameter("xt", [E, R], F32R, isOutput=False)
    it = nc.declare_dram_parameter("it", [E, R], F32R, isOutput=False)
    w1t = nc.declare_dram_parameter("w1t", [E, E], F32R, isOutput=False)
    w2t = nc.declare_dram_parameter("w2t", [E, E], F32R, isOutput=False)
    wf1t = nc.declare_dram_parameter("wf1t", [E, F], F32R, isOutput=False)
    wf2t = nc.declare_dram_parameter("wf2t", [F, E], F32R, isOutput=False)
    # packed per-partition params: [128, c] with [p, c] = v[c*128+p]
    bias1 = nc.declare_dram_parameter("bias1", [128, CH], F32, isOutput=False)
    bias2 = nc.declare_dram_parameter("bias2", [128, CH], F32, isOutput=False)
    bf1 = nc.declare_dram_parameter("bf1", [128, FCH], F32, isOutput=False)
    bf2 = nc.declare_dram_parameter("bf2", [128, CH], F32, isOutput=False)
    # ln params: 6 groups of CH cols: g1 be1 g2 be2 g3 be3
    lnp = nc.declare_dram_parameter("lnp", [128, 6 * CH], F32, isOutput=False)
    ones_in = nc.declare_dram_parameter("ones_in", [128, 1], F32R, isOutput=False)
    ones1_in = nc.declare_dram_parameter("ones1_in", [1, 128], F32R, isOutput=False)
    # f32r and f32 are bit-identical; declaring the output f32r lets the LN3
    # result DMA straight out without a cast.
    ot = nc.declare_dram_parameter("ot", [E, R], F32R, isOutput=True)

    xtr = xt.rearrange("(c p) r -> p c r", p=128)
    itr = it.rearrange("(c p) r -> p c r", p=128)
    otr = ot.rearrange("(c p) r -> p c r", p=128)
    w1r = w1t.rearrange("(c p) m -> p c m", p=128)
    w2r = w2t.rearrange("(c p) m -> p c m", p=128)
    wf1r = wf1t.rearrange("(k p) (g j) -> g p k j", p=128, g=NGRP)
    wf2r = wf2t.rearrange("(k p) m -> p k m", p=128)

    with nc.allow_low_precision("f32r matmul pipeline; accumulation is f32 psum"), \
         tile.TileContext(nc) as tc:
        from contextlib import ExitStack

        with tc.tile_pool(name="dram", bufs=1, space="DRAM") as dram, \
             tc.tile_pool(name="const", bufs=1) as const:
            hbuf = dram.tile([128, FCH, R], F32R)
            cbuf = dram.tile([128, CH, R], F32R)

            b1sb = const.tile([128, CH], F32)
            b2sb = const.tile([128, CH], F32)
            bf1sb = const.tile([128, FCH], F32)
            bf2sb = const.tile([128, CH], F32)
            lnsb = const.tile([128, 6 * CH], F32)
            ones128 = const.tile([128, 1], F32R)
            ones1 = const.tile([1, 128], F32R)
            epsb = const.tile([1, 1], F32)
            zerob = const.tile([128, 1], F32)
            nc.gpsimd.dma_start(out=b1sb[:], in_=bias1[:])
            nc.gpsimd.dma_start(out=b2sb[:], in_=bias2[:])
            nc.gpsimd.dma_start(out=bf1sb[:], in_=bf1[:])
            nc.gpsimd.dma_start(out=bf2sb[:], in_=bf2[:])
            nc.gpsimd.dma_start(out=lnsb[:], in_=lnp[:])
            nc.gpsimd.dma_start(out=ones128[:], in_=ones_in[:])
            nc.gpsimd.dma_start(out=ones1[:], in_=ones1_in[:])
            nc.vector.memset(epsb[:], 1e-5)
            nc.vector.memset(zerob[:], 0.0)

            def layer_norm(ctx_pools, r_t, width, ln_idx, out_t):
                """LN over features of r_t [128, CH, width] -> out_t (may alias).

                Destroys r_t. ln_idx selects g/be columns in lnsb.
                ctx_pools = (sqp, stp, ps_st, ps_bc)
                PE does the feature-dim sums + broadcasts; DVE does centering;
                ACT does squaring and the final g/be scale-bias apply.
                """
                sqp, stp, ps_st, ps_bc = ctx_pools
                g_col = lnsb[:, 2 * ln_idx * CH: (2 * ln_idx + 1) * CH]
                be_col = lnsb[:, (2 * ln_idx + 1) * CH: (2 * ln_idx + 2) * CH]
                s_ps = ps_st.tile([1, width], F32, tag="s_ps")
                q_ps = ps_st.tile([1, width], F32, tag="q_ps")
                for m in range(CH):
                    nc.tensor.matmul(s_ps[:], ones128[:], r_t[:, m, :],
                                     start=(m == 0), stop=(m == CH - 1))
                for m in range(CH):
                    sq = sqp.tile([128, width], F32R, tag="sq")
                    nc.scalar.activation(out=sq[:], in_=r_t[:, m, :], func=AF.Square,
                                         bias=zerob[:])
                    nc.tensor.matmul(q_ps[:], ones128[:], sq[:],
                                     start=(m == 0), stop=(m == CH - 1))
                mu_t = stp.tile([1, width], F32R, tag="mu")
                var_t = stp.tile([1, width], F32, tag="var")
                rstd_t = stp.tile([1, width], F32R, tag="rstd")
                musq = stp.tile([1, width], F32, tag="musq")
                nc.vector.tensor_scalar(out=mu_t[:], in0=s_ps[:], scalar1=1.0 / E,
                                        scalar2=None, op0=OP.mult)
                nc.vector.tensor_scalar(out=var_t[:], in0=q_ps[:], scalar1=1.0 / E,
                                        scalar2=None, op0=OP.mult)
                nc.vector.tensor_tensor(out=musq[:], in0=mu_t[:], in1=mu_t[:], op=OP.mult)
                nc.vector.tensor_tensor(out=var_t[:], in0=var_t[:], in1=musq[:], op=OP.subtract)
                nc.scalar.activation(out=var_t[:], in_=var_t[:], func=AF.Sqrt, bias=epsb[:])
                nc.vector.reciprocal(out=rstd_t[:], in_=var_t[:])
                mu_b = ps_bc.tile([128, width], F32, tag="mu_b")
                rstd_b = ps_bc.tile([128, width], F32, tag="rstd_b")
                nc.tensor.matmul(mu_b[:], ones1[:], mu_t[:], start=True, stop=True)
                nc.tensor.matmul(rstd_b[:], ones1[:], rstd_t[:], start=True, stop=True)
                for m in range(CH):
                    nc.vector.tensor_tensor(out=r_t[:, m, :], in0=r_t[:, m, :],
                                            in1=mu_b[:], op=OP.subtract)
                    nc.vector.tensor_tensor(out=r_t[:, m, :], in0=r_t[:, m, :],
                                            in1=rstd_b[:], op=OP.mult)
                    nc.scalar.activation(out=out_t[:, m, :], in_=r_t[:, m, :],
                                         func=AF.Identity,
                                         scale=g_col[:, m:m + 1],
                                         bias=be_col[:, m:m + 1])

            for rep in range(repeats):
                from contextlib import ExitStack as _ES
                with _ES() as abc:
                    # pimg holds img tiles through AB, which become the c tiles
                    # consumed by phase C (no DRAM round-trip).
                    pimg = abc.enter_context(tc.tile_pool(name="pimg", bufs=NT))

                    # ------------ Phase AB: att1+LN1+att2+LN2 -> c ------------
                    with _ES() as ab:
                        wab = ab.enter_context(tc.tile_pool(name="wab", bufs=1))
                        px = ab.enter_context(tc.tile_pool(name="px", bufs=2))
                        pit = ab.enter_context(tc.tile_pool(name="pit", bufs=2))
                        pr = ab.enter_context(tc.tile_pool(name="pr", bufs=1))
                        sqp = ab.enter_context(tc.tile_pool(name="sqp", bufs=2))
                        stp = ab.enter_context(tc.tile_pool(name="stp", bufs=1))
                        psA = ab.enter_context(tc.tile_pool(name="psA", bufs=4, space="PSUM"))
                        ps_st = ab.enter_context(tc.tile_pool(name="ps_st", bufs=1, space="PSUM"))
                        ps_bc = ab.enter_context(tc.tile_pool(name="ps_bc", bufs=1, space="PSUM"))
                        lnpools = (sqp, stp, ps_st, ps_bc)

                        w1sb = wab.tile([128, CH, E], F32R)
                        w2sb = wab.tile([128, CH, E], F32R)
                        # split weight loads so the first matmuls start sooner
                        HCH = CH // 2
                        nc.sync.dma_start(out=w1sb[:, :HCH, :], in_=w1r[:, :HCH, :])
                        nc.sync.dma_start(out=w1sb[:, HCH:, :], in_=w1r[:, HCH:, :])
                        nc.sync.dma_start(out=w2sb[:, :HCH, :], in_=w2r[:, :HCH, :])
                        nc.sync.dma_start(out=w2sb[:, HCH:, :], in_=w2r[:, HCH:, :])

                        imgs = []
                        for n in range(NT):
                            sl = slice(n * N, (n + 1) * N)
                            xt_t = px.tile([128, CH, N], F32R, tag="xt_t")
                            nc.sync.dma_start(out=xt_t[:], in_=xtr[:, :, sl])

                            r1 = pr.tile([128, CH, N], F32R, tag="r")
                            for m in range(CH):
                                acc = psA.tile([128, N], F32, tag="acc")
                                for k in range(CH):
                                    nc.tensor.matmul(acc[:], w1sb[:, k, ts(m, 128)],
                                                     xt_t[:, k, :],
                                                     start=(k == 0), stop=(k == CH - 1))
                                itc = pit.tile([128, N], F32R, tag="itc")
                                nc.sync.dma_start(out=itc[:], in_=itr[:, m, sl])
                                nc.vector.scalar_tensor_tensor(
                                    out=r1[:, m, :], in0=acc[:], scalar=b1sb[:, m:m + 1],
                                    in1=itc[:], op0=OP.add, op1=OP.add)
                            img = pimg.tile([128, CH, N], F32R, tag="img")
                            layer_norm(lnpools, r1, N, 0, img)

                            r2 = pr.tile([128, CH, N], F32R, tag="r")
                            for m in range(CH):
                                acc = psA.tile([128, N], F32, tag="acc")
                                for k in range(CH):
                                    nc.tensor.matmul(acc[:], w2sb[:, k, ts(m, 128)],
                                                     img[:, k, :],
                                                     start=(k == 0), stop=(k == CH - 1))
                                nc.vector.scalar_tensor_tensor(
                                    out=r2[:, m, :], in0=acc[:], scalar=b2sb[:, m:m + 1],
                                    in1=xt_t[:, m, :], op0=OP.add, op1=OP.add)
                            # LN2 -> txt2 (into r2), then c = txt2 + img (into img)
                            layer_norm(lnpools, r2, N, 1, r2)
                            for m in range(CH):
                                nc.vector.tensor_tensor(out=img[:, m, :], in0=r2[:, m, :],
                                                        in1=img[:, m, :], op=OP.add)
                            nc.sync.dma_start(out=cbuf[:, :, sl], in_=img[:])
                            imgs.append(img)

                    # ------------ Phase C: h = gelu(wf1 @ c + bf1) ------------
                    with _ES() as pc:
                        pw1 = pc.enter_context(tc.tile_pool(name="pw1", bufs=2))
                        ph = pc.enter_context(tc.tile_pool(name="ph", bufs=2))
                        psC = pc.enter_context(tc.tile_pool(name="psC", bufs=4, space="PSUM"))

                        for g in range(NGRP):
                            wg = pw1.tile([128, CH, E], F32R, tag="wg")
                            nc.sync.dma_start(out=wg[:], in_=wf1r[g])
                            for n in range(NT):
                                hst = ph.tile([128, CH, N], F32R, tag="hst")
                                for mj in range(CH):
                                    acc = psC.tile([128, N], F32, tag="accC")
                                    for k in range(CH):
                                        nc.tensor.matmul(acc[:], wg[:, k, ts(mj, 128)],
                                                         imgs[n][:, k, :],
                                                         start=(k == 0), stop=(k == CH - 1))
                                    nc.scalar.activation(hst[:, mj, :], acc[:],
                                                         getattr(AF, _GELU_FUNC),
                                                         bias=bf1sb[:, g * CH + mj: g * CH + mj + 1])
                                nc.sync.dma_start(
                                    out=hbuf[:, g * CH:(g + 1) * CH, n * N:(n + 1) * N],
                                    in_=hst[:])

                # ------------ Phase D: ffn2 + residual + LN3 ------------
                # Batch halves: h half resident (16MB), wf2 streamed per output
                # chunk (re-read once per half), all matmuls at N=512.
                with ExitStack() as pd:
                    phD = pd.enter_context(tc.tile_pool(name="phD", bufs=1))
                    pwm = pd.enter_context(tc.tile_pool(name="pwm", bufs=2))
                    pcD = pd.enter_context(tc.tile_pool(name="pcD", bufs=2))
                    sqpD = pd.enter_context(tc.tile_pool(name="sqpD", bufs=2))
                    stpD = pd.enter_context(tc.tile_pool(name="stpD", bufs=1))
                    psD = pd.enter_context(tc.tile_pool(name="psD", bufs=4, space="PSUM"))
                    ps_stD = pd.enter_context(tc.tile_pool(name="ps_stD", bufs=1, space="PSUM"))
                    ps_bcD = pd.enter_context(tc.tile_pool(name="ps_bcD", bufs=1, space="PSUM"))
                    lnpoolsD = (sqpD, stpD, ps_stD, ps_bcD)

                    HB = R // 2            # 1024 cols per half
                    NTH = HB // N          # 2 tiles per half
                    for half in range(2):
                        hsl = slice(half * HB, (half + 1) * HB)
                        hh = phD.tile([128, FCH, HB], F32R, tag="hh")
                        # split the 16MB load into 4 piece DMAs
                        for piece in range(4):
                            pk = slice(piece * (FCH // 4), (piece + 1) * (FCH // 4))
                            nc.sync.dma_start(out=hh[:, pk, :], in_=hbuf[:, pk, hsl])
                        chs = []
                        for nn in range(NTH):
                            ch = pcD.tile([128, CH, N], F32R, tag="ch")
                            nc.sync.dma_start(
                                out=ch[:],
                                in_=cbuf[:, :, half * HB + nn * N: half * HB + (nn + 1) * N])
                            chs.append(ch)
                        for m in range(CH):
                            wm = pwm.tile([128, FCH, 128], F32R, tag="wm")
                            nc.sync.dma_start(out=wm[:], in_=wf2r[:, :, ts(m, 128)])
                            for nn in range(NTH):
                                acc = psD.tile([128, N], F32, tag="accD")
                                for k in range(FCH):
                                    nc.tensor.matmul(acc[:], wm[:, k, :],
                                                     hh[:, k, nn * N:(nn + 1) * N],
                                                     start=(k == 0), stop=(k == FCH - 1))
                                nc.vector.scalar_tensor_tensor(
                                    out=chs[nn][:, m, :], in0=acc[:],
                                    scalar=bf2sb[:, m:m + 1],
                                    in1=chs[nn][:, m, :], op0=OP.add, op1=OP.add)
                        for nn in range(NTH):
                            osl = slice(half * HB + nn * N, half * HB + (nn + 1) * N)
                            layer_norm(lnpoolsD, chs[nn], N, 2, chs[nn])
                            nc.sync.dma_start(out=otr[:, :, osl], in_=chs[nn][:])

    nc.finalize()
    return nc


def _build(repeats=1):
    from concourse import bacc

    nc = bacc.Bacc()
    return _emit_program(nc, repeats=repeats)


def _make_exec(nc, n_cores=NCORES):
    """Cached jitted SPMD executor, mirroring run_bass_via_pjrt's multi-core
    branch so repeated calls reuse the compiled NEFF."""
    import jax
    import concourse.mybir as mybir
    from concourse import bass2jax
    from jax.experimental.shard_map import shard_map
    from jax.sharding import Mesh, PartitionSpec

    bass2jax.install_neuronx_cc_hook()

    partition_name = nc.partition_id_tensor.name if nc.partition_id_tensor else None
    in_names, out_names, out_avals, zero_shapes = [], [], [], []
    for alloc in nc.m.functions[0].allocations:
        if not isinstance(alloc, mybir.MemoryLocationSet):
            continue
        name = alloc.memorylocations[0].name
        if alloc.kind == "ExternalInput":
            if name != partition_name:
                in_names.append(name)
        elif alloc.kind == "ExternalOutput":
            out_names.append(name)
            shape = tuple(alloc.tensor_shape)
            dtype = mybir.dt.np(alloc.dtype)
            out_avals.append(jax.core.ShapedArray(shape, dtype))
            zero_shapes.append((shape, dtype))
    n_params = len(in_names)
    n_outs = len(out_names)
    all_names = in_names + out_names
    if partition_name is not None:
        all_names = all_names + [partition_name]

    def _body(*args):
        operands = list(args)
        if partition_name is not None:
            operands.append(bass2jax.partition_id_tensor())
        outs = bass2jax._bass_exec_p.bind(
            *operands,
            out_avals=tuple(out_avals),
            in_names=tuple(all_names),
            out_names=tuple(out_names),
            lowering_input_output_aliases=(),
            sim_require_finite=True,
            sim_require_nnan=True,
            nc=nc,
        )
        return tuple(outs)

    devices = jax.devices()[:n_cores]
    mesh = Mesh(np.asarray(devices), ("core",))
    in_specs = (PartitionSpec("core"),) * (n_params + n_outs)
    out_specs = (PartitionSpec("core"),) * n_outs
    donate = tuple(range(n_params, n_params + n_outs))
    sharded = jax.jit(
        shard_map(_body, mesh=mesh, in_specs=in_specs, out_specs=out_specs,
                  check_rep=False),
        donate_argnums=donate, keep_unused=True)

    def run(in_maps):
        concat_in = [
            np.concatenate([np.asarray(in_maps[c][nm]) for c in range(n_cores)], axis=0)
            for nm in in_names
        ]
        concat_zeros = [
            np.zeros((n_cores * s[0],) + tuple(s[1:]), dt) for (s, dt) in zero_shapes
        ]
        out_arrs = sharded(*concat_in, *concat_zeros)
        out_arrs = [np.asarray(a) for a in out_arrs]
        return [
            {nm: out_arrs[i].reshape(n_cores, *out_avals[i].shape)[c]
             for i, nm in enumerate(out_names)}
            for c in range(n_cores)
        ]

    run.in_names = in_names
    run.out_names = out_names
    run.sharded = sharded
    run.n_cores = n_cores
    run.out_avals = out_avals
    run.zero_shapes = zero_shapes
    run.body = _body
    run.mesh = mesh
    run.in_specs = in_specs
    run.out_specs = out_specs
    run.nc = nc
    return run


def _pack_pp(v, ch):
    """bias vector [ch*128] -> per-partition [128, ch]."""
    return np.ascontiguousarray(v.reshape(ch, 128).T.astype(np.float32))


def prepare_in_maps(img_feat, txt_feat, w_in1, b_in1, w_out1, b_out1,
                    w_in2, b_in2, w_out2, b_out2,
                    g1, be1, g2, be2, g3, be3,
                    w_ffn1, b_ffn1, w_ffn2, b_ffn2):
    f32 = np.float32
    img = np.asarray(img_feat, f32)
    txt = np.asarray(txt_feat, f32)
    w_in1 = np.asarray(w_in1, f32); b_in1 = np.asarray(b_in1, f32)
    w_out1 = np.asarray(w_out1, f32); b_out1 = np.asarray(b_out1, f32)
    w_in2 = np.asarray(w_in2, f32); b_in2 = np.asarray(b_in2, f32)
    w_out2 = np.asarray(w_out2, f32); b_out2 = np.asarray(b_out2, f32)
    w_ffn1 = np.asarray(w_ffn1, f32); b_ffn1 = np.asarray(b_ffn1, f32)
    w_ffn2 = np.asarray(w_ffn2, f32); b_ffn2 = np.asarray(b_ffn2, f32)

    wv1 = w_in1[2 * E:]
    bv1 = b_in1[2 * E:]
    W1 = w_out1 @ wv1                      # att1 == txt @ W1.T + b1
    b1 = w_out1 @ bv1 + b_out1
    wv2 = w_in2[2 * E:]
    bv2 = b_in2[2 * E:]
    W2 = w_out2 @ wv2
    b2 = w_out2 @ bv2 + b_out2

    lnp = np.concatenate([
        _pack_pp(np.asarray(v, f32), CH)
        for v in (g1, be1, g2, be2, g3, be3)], axis=1)

    shared = {
        "w1t": np.ascontiguousarray(W1.T),
        "w2t": np.ascontiguousarray(W2.T),
        "wf1t": np.ascontiguousarray(w_ffn1.T),
        "wf2t": np.ascontiguousarray(w_ffn2.T),
        "bias1": _pack_pp(b1, CH),
        "bias2": _pack_pp(b2, CH),
        "bf1": _pack_pp(b_ffn1, FCH),
        "bf2": _pack_pp(b_ffn2, CH),
        "lnp": lnp,
        "ones_in": np.ones((128, 1), f32),
        "ones1_in": np.ones((1, 128), f32),
    }
    in_maps = []
    for c in range(NCORES):
        sh = slice(c * R, (c + 1) * R)
        m = dict(shared)
        m["xt"] = np.ascontiguousarray(txt[sh].T)
        m["it"] = np.ascontiguousarray(img[sh].T)
        in_maps.append(m)
    return in_maps


def get_runner():
    global _RUNNER
    if _RUNNER is None:
        nc = _build()
        _RUNNER = _make_exec(nc)
    return _RUNNER


def kernel(**inputs) -> np.ndarray:
    run = get_runner()
    in_maps = prepare_in_maps(**inputs)
    results = run(in_maps)
    out = np.empty((B, E), np.float32)
    for c in range(NCORES):
        out[c * R:(c + 1) * R] = results[c]["ot"].T
    return out


# revision 9
# speedup vs baseline: 1.4455x; 1.1525x over previous
"""CrossAttentionFusion kernel for 8 Trainium2 NeuronCores.

Math (per reference): two seq-len-1 cross-attention blocks (each reduces to
out_proj(v_proj(x)) = one fused E x E matmul), residual+LN after each, then a
4E FFN with exact-erf GELU and a final residual+LN.

Strategy:
  - Pure data parallel over the batch (16384 rows -> 2048 rows per core).
  - Feature-major ("transposed") activations on device: tiles are
    [128 features, batch] so every matmul is lhsT(=W.T chunk).T @ x.T with no
    on-device transposes. LayerNorm reductions over features run on the PE
    (ones-vector matmuls) with K=1 broadcast matmuls for mean/rstd.
  - f32r (TF32-like fast fp32) matmuls: 4x the plain-fp32 rate.
  - Attention pairs are fused on the host: W1 = w_out1 @ wv1, b1 = w_out1 @
    bv1 + b_out1 (exact algebra, seq_len==1).
  - FFN hidden h [4096 x batch] is spilled through DRAM between ffn1/ffn2.
"""

import os
import sys

import numpy as np

sys.path.insert(0, "/opt/trn_rl_repo")

E = 1024
B = 16384
NCORES = 8
R = B // NCORES          # rows per core
CH = E // 128            # feature chunks (8)
F = 4 * E                # ffn hidden (4096)
FCH = F // 128           # ffn hidden chunks (32)
NGRP = 4                 # ffn1 weight pieces (each 1024 wide)
N = 512                  # batch tile for phases AB/C
NT = R // N              # 4
N2 = 256                 # batch tile for phase D
NH = R // N2             # 8

# CoreSim does not implement Gelu; tests may set KERNEL_GELU=Tanh for
# structural sim checks. Hardware always uses the real (erf) Gelu.
_GELU_FUNC = os.environ.get("KERNEL_GELU", "Gelu")
# dtype for the FFN hidden spill + ffn2 weights: "f32r" (accurate) or "bf16"
# (half the DMA traffic for h/wf2; ~2x less precise ffn2)
_H_DT = os.environ.get("KERNEL_HDT", "bf16")

_RUNNER = None


def _emit_program(nc, repeats=1):
    import concourse.bass as bass
    import concourse.mybir as mybir
    import concourse.tile as tile

    F32 = mybir.dt.float32
    F32R = mybir.dt.float32r
    HDT = mybir.dt.bfloat16 if _H_DT == "bf16" else F32R
    AF = mybir.ActivationFunctionType
    OP = mybir.AluOpType
    ts = bass.ts

    xt = nc.declare_dram_parameter("xt", [E, R], F32R, isOutput=False)
    it = nc.declare_dram_parameter("it", [E, R], F32R, isOutput=False)
    w1t = nc.declare_dram_parameter("w1t", [E, E], F32R, isOutput=False)
    w2t = nc.declare_dram_parameter("w2t", [E, E], F32R, isOutput=False)
    wf1t = nc.declare_dram_parameter("wf1t", [E, F], F32R, isOutput=False)
    wf2t = nc.declare_dram_parameter("wf2t", [F, E], HDT, isOutput=False)
    # packed per-partition params: [128, c] with [p, c] = v[c*128+p]
    bias1 = nc.declare_dram_parameter("bias1", [128, CH], F32, isOutput=False)
    bias2 = nc.declare_dram_parameter("bias2", [128, CH], F32, isOutput=False)
    bf1 = nc.declare_dram_parameter("bf1", [128, FCH], F32, isOutput=False)
    bf2 = nc.declare_dram_parameter("bf2", [128, CH], F32, isOutput=False)
    # ln params: 6 groups of CH cols: g1 be1 g2 be2 g3 be3
    lnp = nc.declare_dram_parameter("lnp", [128, 6 * CH], F32, isOutput=False)
    ones_in = nc.declare_dram_parameter("ones_in", [128, 1], F32R, isOutput=False)
    ones1_in = nc.declare_dram_parameter("ones1_in", [1, 128], F32R, isOutput=False)
    # f32r and f32 are bit-identical; declaring the output f32r lets the LN3
    # result DMA straight out without a cast.
    ot = nc.declare_dram_parameter("ot", [E, R], F32R, isOutput=True)

    xtr = xt.rearrange("(c p) r -> p c r", p=128)
    itr = it.rearrange("(c p) r -> p c r", p=128)
    otr = ot.rearrange("(c p) r -> p c r", p=128)
    w1r = w1t.rearrange("(c p) m -> p c m", p=128)
    w2r = w2t.rearrange("(c p) m -> p c m", p=128)
    wf1r = wf1t.rearrange("(k p) (g j) -> g p k j", p=128, g=NGRP)
    wf2r = wf2t.rearrange("(k p) m -> p k m", p=128)

    with nc.allow_low_precision("f32r matmul pipeline; accumulation is f32 psum"), \
         tile.TileContext(nc) as tc:
        from contextlib import ExitStack

        with tc.tile_pool(name="dram", bufs=1, space="DRAM") as dram, \
             tc.tile_pool(name="const", bufs=1) as const:
            hbuf = dram.tile([128, FCH, R], HDT)
            cbuf = dram.tile([128, CH, R], F32R)

            b1sb = const.tile([128, CH], F32)
            b2sb = const.tile([128, CH], F32)
            bf1sb = const.tile([128, FCH], F32)
            bf2sb = const.tile([128, CH], F32)
            lnsb = const.tile([128, 6 * CH], F32)
            ones128 = const.tile([128, 1], F32R)
            ones1 = const.tile([1, 128], F32R)
            epsb = const.tile([1, 1], F32)
            zerob = const.tile([128, 1], F32)
            nc.gpsimd.dma_start(out=b1sb[:], in_=bias1[:])
            nc.gpsimd.dma_start(out=b2sb[:], in_=bias2[:])
            nc.gpsimd.dma_start(out=bf1sb[:], in_=bf1[:])
            nc.gpsimd.dma_start(out=bf2sb[:], in_=bf2[:])
            nc.gpsimd.dma_start(out=lnsb[:], in_=lnp[:])
            nc.gpsimd.dma_start(out=ones128[:], in_=ones_in[:])
            nc.gpsimd.dma_start(out=ones1[:], in_=ones1_in[:])
            nc.vector.memset(epsb[:], 1e-5)
            nc.vector.memset(zerob[:], 0.0)

            def layer_norm(ctx_pools, r_t, width, ln_idx, out_t):
                """LN over features of r_t [128, CH, width] -> out_t (may alias).

                Destroys r_t. ln_idx selects g/be columns in lnsb.
                ctx_pools = (sqp, stp, ps_st, ps_bc)
                PE does the feature-dim sums + broadcasts; DVE does centering;
                ACT does squaring and the final g/be scale-bias apply.
                """
                sqp, stp, ps_st, ps_bc = ctx_pools
                g_col = lnsb[:, 2 * ln_idx * CH: (2 * ln_idx + 1) * CH]
                be_col = lnsb[:, (2 * ln_idx + 1) * CH: (2 * ln_idx + 2) * CH]
                s_ps = ps_st.tile([1, width], F32, tag="s_ps")
                q_ps = ps_st.tile([1, width], F32, tag="q_ps")
                for m in range(CH):
                    nc.tensor.matmul(s_ps[:], ones128[:], r_t[:, m, :],
                                     start=(m == 0), stop=(m == CH - 1))
                for m in range(CH):
                    sq = sqp.tile([128, width], F32R, tag="sq")
                    nc.scalar.activation(out=sq[:], in_=r_t[:, m, :], func=AF.Square,
                                         bias=zerob[:])
                    nc.tensor.matmul(q_ps[:], ones128[:], sq[:],
                                     start=(m == 0), stop=(m == CH - 1))
                mu_t = stp.tile([1, width], F32R, tag="mu")
                var_t = stp.tile([1, width], F32, tag="var")
                rstd_t = stp.tile([1, width], F32R, tag="rstd")
                musq = stp.tile([1, width], F32, tag="musq")
                nc.vector.tensor_scalar(out=mu_t[:], in0=s_ps[:], scalar1=1.0 / E,
                                        scalar2=None, op0=OP.mult)
                nc.vector.tensor_scalar(out=var_t[:], in0=q_ps[:], scalar1=1.0 / E,
                                        scalar2=None, op0=OP.mult)
                nc.vector.tensor_tensor(out=musq[:], in0=mu_t[:], in1=mu_t[:], op=OP.mult)
                nc.vector.tensor_tensor(out=var_t[:], in0=var_t[:], in1=musq[:], op=OP.subtract)
                nc.scalar.activation(out=var_t[:], in_=var_t[:], func=AF.Sqrt, bias=epsb[:])
                nc.vector.reciprocal(out=rstd_t[:], in_=var_t[:])
                mu_b = ps_bc.tile([128, width], F32, tag="mu_b")
                rstd_b = ps_bc.tile([128, width], F32, tag="rstd_b")
                nc.tensor.matmul(mu_b[:], ones1[:], mu_t[:], start=True, stop=True)
                nc.tensor.matmul(rstd_b[:], ones1[:], rstd_t[:], start=True, stop=True)
                for m in range(CH):
                    nc.vector.tensor_tensor(out=r_t[:, m, :], in0=r_t[:, m, :],
                                            in1=mu_b[:], op=OP.subtract)
                    nc.vector.tensor_tensor(out=r_t[:, m, :], in0=r_t[:, m, :],
                                            in1=rstd_b[:], op=OP.mult)
                    nc.scalar.activation(out=out_t[:, m, :], in_=r_t[:, m, :],
                                         func=AF.Identity,
                                         scale=g_col[:, m:m + 1],
                                         bias=be_col[:, m:m + 1])

            for rep in range(repeats):
                from contextlib import ExitStack as _ES
                with _ES() as abc:
                    # pimg holds img tiles through AB, which become the c tiles
                    # consumed by phase C (no DRAM round-trip).
                    pimg = abc.enter_context(tc.tile_pool(name="pimg", bufs=NT))

                    # ------------ Phase AB: att1+LN1+att2+LN2 -> c ------------
                    with _ES() as ab:
                        wab = ab.enter_context(tc.tile_pool(name="wab", bufs=1))
                        px = ab.enter_context(tc.tile_pool(name="px", bufs=2))
                        pit = ab.enter_context(tc.tile_pool(name="pit", bufs=2))
                        pr = ab.enter_context(tc.tile_pool(name="pr", bufs=1))
                        sqp = ab.enter_context(tc.tile_pool(name="sqp", bufs=2))
                        stp = ab.enter_context(tc.tile_pool(name="stp", bufs=1))
                        psA = ab.enter_context(tc.tile_pool(name="psA", bufs=4, space="PSUM"))
                        ps_st = ab.enter_context(tc.tile_pool(name="ps_st", bufs=1, space="PSUM"))
                        ps_bc = ab.enter_context(tc.tile_pool(name="ps_bc", bufs=1, space="PSUM"))
                        lnpools = (sqp, stp, ps_st, ps_bc)

                        w1sb = wab.tile([128, CH, E], F32R)
                        w2sb = wab.tile([128, CH, E], F32R)
                        # split weight loads so the first matmuls start sooner
                        HCH = CH // 2
                        nc.sync.dma_start(out=w1sb[:, :HCH, :], in_=w1r[:, :HCH, :])
                        nc.sync.dma_start(out=w1sb[:, HCH:, :], in_=w1r[:, HCH:, :])
                        nc.sync.dma_start(out=w2sb[:, :HCH, :], in_=w2r[:, :HCH, :])
                        nc.sync.dma_start(out=w2sb[:, HCH:, :], in_=w2r[:, HCH:, :])

                        imgs = []
                        for n in range(NT):
                            sl = slice(n * N, (n + 1) * N)
                            xt_t = px.tile([128, CH, N], F32R, tag="xt_t")
                            nc.sync.dma_start(out=xt_t[:], in_=xtr[:, :, sl])

                            r1 = pr.tile([128, CH, N], F32R, tag="r")
                            for m in range(CH):
                                acc = psA.tile([128, N], F32, tag="acc")
                                for k in range(CH):
                                    nc.tensor.matmul(acc[:], w1sb[:, k, ts(m, 128)],
                                                     xt_t[:, k, :],
                                                     start=(k == 0), stop=(k == CH - 1))
                                itc = pit.tile([128, N], F32R, tag="itc")
                                nc.sync.dma_start(out=itc[:], in_=itr[:, m, sl])
                                nc.vector.scalar_tensor_tensor(
                                    out=r1[:, m, :], in0=acc[:], scalar=b1sb[:, m:m + 1],
                                    in1=itc[:], op0=OP.add, op1=OP.add)
                            img = pimg.tile([128, CH, N], F32R, tag="img")
                            layer_norm(lnpools, r1, N, 0, img)

                            r2 = pr.tile([128, CH, N], F32R, tag="r")
                            for m in range(CH):
                                acc = psA.tile([128, N], F32, tag="acc")
                                for k in range(CH):
                                    nc.tensor.matmul(acc[:], w2sb[:, k, ts(m, 128)],
                                                     img[:, k, :],
                                                     start=(k == 0), stop=(k == CH - 1))
                                nc.vector.scalar_tensor_tensor(
                                    out=r2[:, m, :], in0=acc[:], scalar=b2sb[:, m:m + 1],
                                    in1=xt_t[:, m, :], op0=OP.add, op1=OP.add)
                            # LN2 -> txt2 (into r2), then c = txt2 + img (into img)
                            layer_norm(lnpools, r2, N, 1, r2)
                            for m in range(CH):
                                nc.vector.tensor_tensor(out=img[:, m, :], in0=r2[:, m, :],
                                                        in1=img[:, m, :], op=OP.add)
                            nc.sync.dma_start(out=cbuf[:, :, sl], in_=img[:])
                            imgs.append(img)

                    # ------------ Phase C: h = gelu(wf1 @ c + bf1) ------------
                    with _ES() as pc:
                        pw1 = pc.enter_context(tc.tile_pool(name="pw1", bufs=2))
                        ph = pc.enter_context(tc.tile_pool(name="ph", bufs=2))
                        psC = pc.enter_context(tc.tile_pool(name="psC", bufs=4, space="PSUM"))

                        for g in range(NGRP):
                            wg = pw1.tile([128, CH, E], F32R, tag="wg")
                            nc.sync.dma_start(out=wg[:], in_=wf1r[g])
                            for n in range(NT):
                                hst = ph.tile([128, CH, N], HDT, tag="hst")
                                for mj in range(CH):
                                    acc = psC.tile([128, N], F32, tag="accC")
                                    for k in range(CH):
                                        nc.tensor.matmul(acc[:], wg[:, k, ts(mj, 128)],
                                                         imgs[n][:, k, :],
                                                         start=(k == 0), stop=(k == CH - 1))
                                    nc.scalar.activation(hst[:, mj, :], acc[:],
                                                         getattr(AF, _GELU_FUNC),
                                                         bias=bf1sb[:, g * CH + mj: g * CH + mj + 1])
                                nc.sync.dma_start(
                                    out=hbuf[:, g * CH:(g + 1) * CH, n * N:(n + 1) * N],
                                    in_=hst[:])

                # ------------ Phase D: ffn2 + residual + LN3 ------------
                # Batch halves: h half resident (16MB), wf2 streamed per output
                # chunk (re-read once per half), all matmuls at N=512.
                with ExitStack() as pd:
                    phD = pd.enter_context(tc.tile_pool(
                        name="phD", bufs=2 if _H_DT == "bf16" else 1))
                    pwm = pd.enter_context(tc.tile_pool(name="pwm", bufs=2))
                    pcD = pd.enter_context(tc.tile_pool(name="pcD", bufs=2))
                    sqpD = pd.enter_context(tc.tile_pool(name="sqpD", bufs=2))
                    stpD = pd.enter_context(tc.tile_pool(name="stpD", bufs=1))
                    psD = pd.enter_context(tc.tile_pool(name="psD", bufs=4, space="PSUM"))
                    ps_stD = pd.enter_context(tc.tile_pool(name="ps_stD", bufs=1, space="PSUM"))
                    ps_bcD = pd.enter_context(tc.tile_pool(name="ps_bcD", bufs=1, space="PSUM"))
                    lnpoolsD = (sqpD, stpD, ps_stD, ps_bcD)

                    HB = R // 2            # 1024 cols per half
                    NTH = HB // N          # 2 tiles per half
                    for half in range(2):
                        hsl = slice(half * HB, (half + 1) * HB)
                        hh = phD.tile([128, FCH, HB], HDT, tag="hh")
                        # split the 16MB load into 4 piece DMAs
                        for piece in range(4):
                            pk = slice(piece * (FCH // 4), (piece + 1) * (FCH // 4))
                            nc.sync.dma_start(out=hh[:, pk, :], in_=hbuf[:, pk, hsl])
                        chs = []
                        for nn in range(NTH):
                            ch = pcD.tile([128, CH, N], F32R, tag="ch")
                            nc.sync.dma_start(
                                out=ch[:],
                                in_=cbuf[:, :, half * HB + nn * N: half * HB + (nn + 1) * N])
                            chs.append(ch)
                        for m in range(CH):
                            wm = pwm.tile([128, FCH, 128], HDT, tag="wm")
                            nc.sync.dma_start(out=wm[:], in_=wf2r[:, :, ts(m, 128)])
                            for nn in range(NTH):
                                acc = psD.tile([128, N], F32, tag="accD")
                                for k in range(FCH):
                                    nc.tensor.matmul(acc[:], wm[:, k, :],
                                                     hh[:, k, nn * N:(nn + 1) * N],
                                                     start=(k == 0), stop=(k == FCH - 1))
                                nc.vector.scalar_tensor_tensor(
                                    out=chs[nn][:, m, :], in0=acc[:],
                                    scalar=bf2sb[:, m:m + 1],
                                    in1=chs[nn][:, m, :], op0=OP.add, op1=OP.add)
                        for nn in range(NTH):
                            osl = slice(half * HB + nn * N, half * HB + (nn + 1) * N)
                            layer_norm(lnpoolsD, chs[nn], N, 2, chs[nn])
                            nc.sync.dma_start(out=otr[:, :, osl], in_=chs[nn][:])

    nc.finalize()
    return nc


def _build(repeats=1):
    from concourse import bacc

    nc = bacc.Bacc()
    return _emit_program(nc, repeats=repeats)


def _make_exec(nc, n_cores=NCORES):
    """Cached jitted SPMD executor, mirroring run_bass_via_pjrt's multi-core
    branch so repeated calls reuse the compiled NEFF."""
    import jax
    import concourse.mybir as mybir
    from concourse import bass2jax
    from jax.experimental.shard_map import shard_map
    from jax.sharding import Mesh, PartitionSpec

    bass2jax.install_neuronx_cc_hook()

    partition_name = nc.partition_id_tensor.name if nc.partition_id_tensor else None
    in_names, out_names, out_avals, zero_shapes = [], [], [], []
    for alloc in nc.m.functions[0].allocations:
        if not isinstance(alloc, mybir.MemoryLocationSet):
            continue
        name = alloc.memorylocations[0].name
        if alloc.kind == "ExternalInput":
            if name != partition_name:
                in_names.append(name)
        elif alloc.kind == "ExternalOutput":
            out_names.append(name)
            shape = tuple(alloc.tensor_shape)
            dtype = mybir.dt.np(alloc.dtype)
            out_avals.append(jax.core.ShapedArray(shape, dtype))
            zero_shapes.append((shape, dtype))
    n_params = len(in_names)
    n_outs = len(out_names)
    all_names = in_names + out_names
    if partition_name is not None:
        all_names = all_names + [partition_name]

    def _body(*args):
        operands = list(args)
        if partition_name is not None:
            operands.append(bass2jax.partition_id_tensor())
        outs = bass2jax._bass_exec_p.bind(
            *operands,
            out_avals=tuple(out_avals),
            in_names=tuple(all_names),
            out_names=tuple(out_names),
            lowering_input_output_aliases=(),
            sim_require_finite=True,
            sim_require_nnan=True,
            nc=nc,
        )
        return tuple(outs)

    devices = jax.devices()[:n_cores]
    mesh = Mesh(np.asarray(devices), ("core",))
    in_specs = (PartitionSpec("core"),) * (n_params + n_outs)
    out_specs = (PartitionSpec("core"),) * n_outs
    donate = tuple(range(n_params, n_params + n_outs))
    sharded = jax.jit(
        shard_map(_body, mesh=mesh, in_specs=in_specs, out_specs=out_specs,
                  check_rep=False),
        donate_argnums=donate, keep_unused=True)

    def run(in_maps):
        concat_in = [
            np.concatenate([np.asarray(in_maps[c][nm]) for c in range(n_cores)], axis=0)
            for nm in in_names
        ]
        concat_zeros = [
            np.zeros((n_cores * s[0],) + tuple(s[1:]), dt) for (s, dt) in zero_shapes
        ]
        out_arrs = sharded(*concat_in, *concat_zeros)
        out_arrs = [np.asarray(a) for a in out_arrs]
        return [
            {nm: out_arrs[i].reshape(n_cores, *out_avals[i].shape)[c]
             for i, nm in enumerate(out_names)}
            for c in range(n_cores)
        ]

    run.in_names = in_names
    run.out_names = out_names
    run.sharded = sharded
    run.n_cores = n_cores
    run.out_avals = out_avals
    run.zero_shapes = zero_shapes
    run.body = _body
    run.mesh = mesh
    run.in_specs = in_specs
    run.out_specs = out_specs
    run.nc = nc
    return run


def _pack_pp(v, ch):
    """bias vector [ch*128] -> per-partition [128, ch]."""
    return np.ascontiguousarray(v.reshape(ch, 128).T.astype(np.float32))


def prepare_in_maps(img_feat, txt_feat, w_in1, b_in1, w_out1, b_out1,
                    w_in2, b_in2, w_out2, b_out2,
                    g1, be1, g2, be2, g3, be3,
                    w_ffn1, b_ffn1, w_ffn2, b_ffn2):
    f32 = np.float32
    img = np.asarray(img_feat, f32)
    txt = np.asarray(txt_feat, f32)
    w_in1 = np.asarray(w_in1, f32); b_in1 = np.asarray(b_in1, f32)
    w_out1 = np.asarray(w_out1, f32); b_out1 = np.asarray(b_out1, f32)
    w_in2 = np.asarray(w_in2, f32); b_in2 = np.asarray(b_in2, f32)
    w_out2 = np.asarray(w_out2, f32); b_out2 = np.asarray(b_out2, f32)
    w_ffn1 = np.asarray(w_ffn1, f32); b_ffn1 = np.asarray(b_ffn1, f32)
    w_ffn2 = np.asarray(w_ffn2, f32); b_ffn2 = np.asarray(b_ffn2, f32)

    wv1 = w_in1[2 * E:]
    bv1 = b_in1[2 * E:]
    W1 = w_out1 @ wv1                      # att1 == txt @ W1.T + b1
    b1 = w_out1 @ bv1 + b_out1
    wv2 = w_in2[2 * E:]
    bv2 = b_in2[2 * E:]
    W2 = w_out2 @ wv2
    b2 = w_out2 @ bv2 + b_out2

    lnp = np.concatenate([
        _pack_pp(np.asarray(v, f32), CH)
        for v in (g1, be1, g2, be2, g3, be3)], axis=1)

    if _H_DT == "bf16":
        import ml_dtypes
        wf2t_host = np.ascontiguousarray(w_ffn2.T.astype(ml_dtypes.bfloat16))
    else:
        wf2t_host = np.ascontiguousarray(w_ffn2.T)
    shared = {
        "w1t": np.ascontiguousarray(W1.T),
        "w2t": np.ascontiguousarray(W2.T),
        "wf1t": np.ascontiguousarray(w_ffn1.T),
        "wf2t": wf2t_host,
        "bias1": _pack_pp(b1, CH),
        "bias2": _pack_pp(b2, CH),
        "bf1": _pack_pp(b_ffn1, FCH),
        "bf2": _pack_pp(b_ffn2, CH),
        "lnp": lnp,
        "ones_in": np.ones((128, 1), f32),
        "ones1_in": np.ones((1, 128), f32),
    }
    in_maps = []
    for c in range(NCORES):
        sh = slice(c * R, (c + 1) * R)
        m = dict(shared)
        m["xt"] = np.ascontiguousarray(txt[sh].T)
        m["it"] = np.ascontiguousarray(img[sh].T)
        in_maps.append(m)
    return in_maps


def get_runner():
    global _RUNNER
    if _RUNNER is None:
        nc = _build()
        _RUNNER = _make_exec(nc)
    return _RUNNER


def kernel(**inputs) -> np.ndarray:
    run = get_runner()
    in_maps = prepare_in_maps(**inputs)
    results = run(in_maps)
    out = np.empty((B, E), np.float32)
    for c in range(NCORES):
        out[c * R:(c + 1) * R] = results[c]["ot"].T
    return out


# revision 12
# speedup vs baseline: 1.5191x; 1.0509x over previous
"""CrossAttentionFusion kernel for 8 Trainium2 NeuronCores.

Math (per reference): two seq-len-1 cross-attention blocks (each reduces to
out_proj(v_proj(x)) = one fused E x E matmul), residual+LN after each, then a
4E FFN with exact-erf GELU and a final residual+LN.

Strategy:
  - Pure data parallel over the batch (16384 rows -> 2048 rows per core).
  - Feature-major ("transposed") activations on device: tiles are
    [128 features, batch] so every matmul is lhsT(=W.T chunk).T @ x.T with no
    on-device transposes. LayerNorm reductions over features run on the PE
    (ones-vector matmuls) with K=1 broadcast matmuls for mean/rstd.
  - f32r (TF32-like fast fp32) matmuls: 4x the plain-fp32 rate.
  - Attention pairs are fused on the host: W1 = w_out1 @ wv1, b1 = w_out1 @
    bv1 + b_out1 (exact algebra, seq_len==1).
  - FFN hidden h [4096 x batch] is spilled through DRAM between ffn1/ffn2.
"""

import os
import sys

import numpy as np

sys.path.insert(0, "/opt/trn_rl_repo")

E = 1024
B = 16384
NCORES = 8
R = B // NCORES          # rows per core
CH = E // 128            # feature chunks (8)
F = 4 * E                # ffn hidden (4096)
FCH = F // 128           # ffn hidden chunks (32)
NGRP = 4                 # ffn1 weight pieces (each 1024 wide)
N = 512                  # batch tile for phases AB/C
NT = R // N              # 4
N2 = 256                 # batch tile for phase D
NH = R // N2             # 8

# CoreSim does not implement Gelu; tests may set KERNEL_GELU=Tanh for
# structural sim checks. Hardware always uses the real (erf) Gelu.
_GELU_FUNC = os.environ.get("KERNEL_GELU", "Gelu")
# dtype for the FFN hidden spill + ffn2 weights: "f32r" (accurate) or "bf16"
# (half the DMA traffic for h/wf2; ~2x less precise ffn2)
_H_DT = os.environ.get("KERNEL_HDT", "bf16")

_RUNNER = None


def _emit_program(nc, repeats=1):
    import concourse.bass as bass
    import concourse.mybir as mybir
    import concourse.tile as tile

    F32 = mybir.dt.float32
    F32R = mybir.dt.float32r
    HDT = mybir.dt.bfloat16 if _H_DT == "bf16" else F32R
    AF = mybir.ActivationFunctionType
    OP = mybir.AluOpType
    ts = bass.ts

    xt = nc.declare_dram_parameter("xt", [E, R], F32R, isOutput=False)
    it = nc.declare_dram_parameter("it", [E, R], F32R, isOutput=False)
    w1t = nc.declare_dram_parameter("w1t", [E, E], F32R, isOutput=False)
    w2t = nc.declare_dram_parameter("w2t", [E, E], F32R, isOutput=False)
    wf1t = nc.declare_dram_parameter("wf1t", [E, F], F32R, isOutput=False)
    wf2t = nc.declare_dram_parameter("wf2t", [F, E], HDT, isOutput=False)
    # packed per-partition params: [128, c] with [p, c] = v[c*128+p]
    bias1 = nc.declare_dram_parameter("bias1", [128, CH], F32, isOutput=False)
    bias2 = nc.declare_dram_parameter("bias2", [128, CH], F32, isOutput=False)
    bf1 = nc.declare_dram_parameter("bf1", [128, FCH], F32, isOutput=False)
    bf2 = nc.declare_dram_parameter("bf2", [128, CH], F32, isOutput=False)
    # ln params: 6 groups of CH cols: g1 be1 g2 be2 g3 be3
    lnp = nc.declare_dram_parameter("lnp", [128, 6 * CH], F32, isOutput=False)
    ones_in = nc.declare_dram_parameter("ones_in", [128, 1], F32R, isOutput=False)
    ones1_in = nc.declare_dram_parameter("ones1_in", [1, 128], F32R, isOutput=False)
    # f32r and f32 are bit-identical; declaring the output f32r lets the LN3
    # result DMA straight out without a cast.
    ot = nc.declare_dram_parameter("ot", [E, R], F32R, isOutput=True)

    xtr = xt.rearrange("(c p) r -> p c r", p=128)
    itr = it.rearrange("(c p) r -> p c r", p=128)
    otr = ot.rearrange("(c p) r -> p c r", p=128)
    w1r = w1t.rearrange("(c p) m -> p c m", p=128)
    w2r = w2t.rearrange("(c p) m -> p c m", p=128)
    wf1r = wf1t.rearrange("(k p) (g j) -> g p k j", p=128, g=NGRP)
    wf2r = wf2t.rearrange("(k p) m -> p k m", p=128)

    with nc.allow_low_precision("f32r matmul pipeline; accumulation is f32 psum"), \
         tile.TileContext(nc) as tc:
        from contextlib import ExitStack

        with tc.tile_pool(name="dram", bufs=1, space="DRAM") as dram, \
             tc.tile_pool(name="const", bufs=1) as const:
            hbuf = dram.tile([128, FCH, R], HDT)
            cbuf = dram.tile([128, CH, R], F32R)

            b1sb = const.tile([128, CH], F32)
            b2sb = const.tile([128, CH], F32)
            bf1sb = const.tile([128, FCH], F32)
            bf2sb = const.tile([128, CH], F32)
            lnsb = const.tile([128, 6 * CH], F32)
            ones128 = const.tile([128, 1], F32R)
            ones1 = const.tile([1, 128], F32R)
            epsb = const.tile([1, 1], F32)
            zerob = const.tile([128, 1], F32)
            nc.gpsimd.dma_start(out=b1sb[:], in_=bias1[:])
            nc.gpsimd.dma_start(out=b2sb[:], in_=bias2[:])
            nc.gpsimd.dma_start(out=bf1sb[:], in_=bf1[:])
            nc.gpsimd.dma_start(out=bf2sb[:], in_=bf2[:])
            nc.gpsimd.dma_start(out=lnsb[:], in_=lnp[:])
            nc.gpsimd.dma_start(out=ones128[:], in_=ones_in[:])
            nc.gpsimd.dma_start(out=ones1[:], in_=ones1_in[:])
            nc.vector.memset(epsb[:], 1e-5)
            nc.vector.memset(zerob[:], 0.0)

            def layer_norm(ctx_pools, r_t, width, ln_idx, out_t):
                """LN over features of r_t [128, CH, width] -> out_t (may alias).

                Destroys r_t. ln_idx selects g/be columns in lnsb.
                ctx_pools = (sqp, stp, ps_st, ps_bc)
                PE does the feature-dim sums + broadcasts; DVE does centering;
                ACT does squaring and the final g/be scale-bias apply.
                """
                sqp, stp, ps_st, ps_bc = ctx_pools
                g_col = lnsb[:, 2 * ln_idx * CH: (2 * ln_idx + 1) * CH]
                be_col = lnsb[:, (2 * ln_idx + 1) * CH: (2 * ln_idx + 2) * CH]
                s_ps = ps_st.tile([1, width], F32, tag="s_ps")
                q_ps = ps_st.tile([1, width], F32, tag="q_ps")
                for m in range(CH):
                    nc.tensor.matmul(s_ps[:], ones128[:], r_t[:, m, :],
                                     start=(m == 0), stop=(m == CH - 1))
                for m in range(CH):
                    sq = sqp.tile([128, width], F32R, tag="sq")
                    nc.scalar.activation(out=sq[:], in_=r_t[:, m, :], func=AF.Square,
                                         bias=zerob[:])
                    nc.tensor.matmul(q_ps[:], ones128[:], sq[:],
                                     start=(m == 0), stop=(m == CH - 1))
                mu_t = stp.tile([1, width], F32R, tag="mu")
                var_t = stp.tile([1, width], F32, tag="var")
                rstd_t = stp.tile([1, width], F32R, tag="rstd")
                musq = stp.tile([1, width], F32, tag="musq")
                nc.vector.tensor_scalar(out=mu_t[:], in0=s_ps[:], scalar1=1.0 / E,
                                        scalar2=None, op0=OP.mult)
                nc.vector.tensor_tensor(out=musq[:], in0=mu_t[:], in1=mu_t[:], op=OP.mult)
                nc.vector.scalar_tensor_tensor(out=var_t[:], in0=q_ps[:],
                                               scalar=1.0 / E, in1=musq[:],
                                               op0=OP.mult, op1=OP.subtract)
                nc.scalar.activation(out=var_t[:], in_=var_t[:], func=AF.Sqrt, bias=epsb[:])
                nc.vector.reciprocal(out=rstd_t[:], in_=var_t[:])
                mu_b = ps_bc.tile([128, width], F32, tag="mu_b")
                rstd_b = ps_bc.tile([128, width], F32, tag="rstd_b")
                nc.tensor.matmul(mu_b[:], ones1[:], mu_t[:], start=True, stop=True)
                nc.tensor.matmul(rstd_b[:], ones1[:], rstd_t[:], start=True, stop=True)
                for m in range(CH):
                    nc.vector.tensor_tensor(out=r_t[:, m, :], in0=r_t[:, m, :],
                                            in1=mu_b[:], op=OP.subtract)
                    nc.vector.tensor_tensor(out=r_t[:, m, :], in0=r_t[:, m, :],
                                            in1=rstd_b[:], op=OP.mult)
                    nc.scalar.activation(out=out_t[:, m, :], in_=r_t[:, m, :],
                                         func=AF.Identity,
                                         scale=g_col[:, m:m + 1],
                                         bias=be_col[:, m:m + 1])

            for rep in range(repeats):
                # ------------ Phase AB: att1+LN1+att2+LN2 -> c ------------
                with ExitStack() as ab:
                    wab = ab.enter_context(tc.tile_pool(name="wab", bufs=1))
                    px = ab.enter_context(tc.tile_pool(name="px", bufs=2))
                    pit = ab.enter_context(tc.tile_pool(name="pit", bufs=2))
                    pr = ab.enter_context(tc.tile_pool(name="pr", bufs=2))
                    pimg = ab.enter_context(tc.tile_pool(name="pimg", bufs=2))
                    sqp = ab.enter_context(tc.tile_pool(name="sqp", bufs=2))
                    stp = ab.enter_context(tc.tile_pool(name="stp", bufs=1))
                    psA = ab.enter_context(tc.tile_pool(name="psA", bufs=4, space="PSUM"))
                    ps_st = ab.enter_context(tc.tile_pool(name="ps_st", bufs=1, space="PSUM"))
                    ps_bc = ab.enter_context(tc.tile_pool(name="ps_bc", bufs=1, space="PSUM"))
                    lnpools = (sqp, stp, ps_st, ps_bc)

                    w1sb = wab.tile([128, CH, E], F32R)
                    w2sb = wab.tile([128, CH, E], F32R)
                    HCH = CH // 2
                    nc.sync.dma_start(out=w1sb[:, :HCH, :], in_=w1r[:, :HCH, :])
                    nc.sync.dma_start(out=w1sb[:, HCH:, :], in_=w1r[:, HCH:, :])
                    nc.sync.dma_start(out=w2sb[:, :HCH, :], in_=w2r[:, :HCH, :])
                    nc.sync.dma_start(out=w2sb[:, HCH:, :], in_=w2r[:, HCH:, :])

                    def attention(wsb, rhs_tile, rhs_split, bias_sb, resid_tile, out_r):
                        """out_r[m] = (wsb.T @ rhs)[m] + bias[m] + resid[m].

                        k-major in two m-groups of 4 so matmuls start as soon
                        as rhs chunk k is available (rhs_split marks per-chunk
                        availability mattering; purely an emission order).
                        """
                        for mg in range(2):
                            accs = []
                            for _mi in range(4):
                                acc_g = psA.tile([128, N], F32, tag="acc", name=f"acc_g{_mi}")
                                accs.append(acc_g)
                            for k in range(CH):
                                for mi in range(4):
                                    m = mg * 4 + mi
                                    nc.tensor.matmul(accs[mi][:], wsb[:, k, ts(m, 128)],
                                                     rhs_tile[:, k, :],
                                                     start=(k == 0), stop=(k == CH - 1))
                            for mi in range(4):
                                m = mg * 4 + mi
                                nc.vector.scalar_tensor_tensor(
                                    out=out_r[:, m, :], in0=accs[mi][:],
                                    scalar=bias_sb[:, m:m + 1],
                                    in1=resid_tile(m), op0=OP.add, op1=OP.add)

                    for n in range(NT):
                        sl = slice(n * N, (n + 1) * N)
                        xt_t = px.tile([128, CH, N], F32R, tag="xt_t")
                        nc.sync.dma_start(out=xt_t[:, :HCH, :], in_=xtr[:, :HCH, sl])
                        nc.sync.dma_start(out=xt_t[:, HCH:, :], in_=xtr[:, HCH:, sl])
                        it_t = pit.tile([128, CH, N], F32R, tag="it_t")
                        nc.sync.dma_start(out=it_t[:, :HCH, :], in_=itr[:, :HCH, sl])
                        nc.sync.dma_start(out=it_t[:, HCH:, :], in_=itr[:, HCH:, sl])

                        r1 = pr.tile([128, CH, N], F32R, tag="r")
                        attention(w1sb, xt_t, True, b1sb, lambda m: it_t[:, m, :], r1)
                        img = pimg.tile([128, CH, N], F32R, tag="img")
                        layer_norm(lnpools, r1, N, 0, img)

                        r2 = pr.tile([128, CH, N], F32R, tag="r")
                        attention(w2sb, img, True, b2sb, lambda m: xt_t[:, m, :], r2)
                        # LN2 -> txt2 (into r2), then c = txt2 + img (into img)
                        layer_norm(lnpools, r2, N, 1, r2)
                        for m in range(CH):
                            nc.vector.tensor_tensor(out=img[:, m, :], in0=r2[:, m, :],
                                                    in1=img[:, m, :], op=OP.add)
                        nc.sync.dma_start(out=cbuf[:, :, sl], in_=img[:])

                # ------------ Phase C: h = gelu(wf1 @ c + bf1) ------------
                with ExitStack() as pc:
                    pcc = pc.enter_context(tc.tile_pool(name="pcc", bufs=NT))
                    pw1 = pc.enter_context(tc.tile_pool(name="pw1", bufs=2))
                    ph = pc.enter_context(tc.tile_pool(name="ph", bufs=2))
                    psC = pc.enter_context(tc.tile_pool(name="psC", bufs=4, space="PSUM"))

                    c_ts = []
                    for n in range(NT):
                        ct = pcc.tile([128, CH, N], F32R, tag="ct")
                        nc.sync.dma_start(out=ct[:], in_=cbuf[:, :, n * N:(n + 1) * N])
                        c_ts.append(ct)
                    for g in range(NGRP):
                        wg = pw1.tile([128, CH, E], F32R, tag="wg")
                        nc.sync.dma_start(out=wg[:], in_=wf1r[g])
                        for n in range(NT):
                            hst = ph.tile([128, CH, N], HDT, tag="hst")
                            for mj in range(CH):
                                acc = psC.tile([128, N], F32, tag="accC")
                                for k in range(CH):
                                    nc.tensor.matmul(acc[:], wg[:, k, ts(mj, 128)],
                                                     c_ts[n][:, k, :],
                                                     start=(k == 0), stop=(k == CH - 1))
                                nc.scalar.activation(hst[:, mj, :], acc[:],
                                                     getattr(AF, _GELU_FUNC),
                                                     bias=bf1sb[:, g * CH + mj: g * CH + mj + 1])
                            nc.sync.dma_start(
                                out=hbuf[:, g * CH:(g + 1) * CH, n * N:(n + 1) * N],
                                in_=hst[:])

                # ------------ Phase D: ffn2 + residual + LN3 ------------
                with ExitStack() as pd:
                    phD = pd.enter_context(tc.tile_pool(
                        name="phD", bufs=2 if _H_DT == "bf16" else 1))
                    pwm = pd.enter_context(tc.tile_pool(name="pwm", bufs=2))
                    pcD = pd.enter_context(tc.tile_pool(name="pcD", bufs=2))
                    sqpD = pd.enter_context(tc.tile_pool(name="sqpD", bufs=2))
                    stpD = pd.enter_context(tc.tile_pool(name="stpD", bufs=2))
                    psD = pd.enter_context(tc.tile_pool(name="psD", bufs=4, space="PSUM"))
                    ps_stD = pd.enter_context(tc.tile_pool(name="ps_stD", bufs=1, space="PSUM"))
                    ps_bcD = pd.enter_context(tc.tile_pool(name="ps_bcD", bufs=1, space="PSUM"))
                    lnpoolsD = (sqpD, stpD, ps_stD, ps_bcD)

                    HB = R // 2            # 1024 cols per half
                    NTH = HB // N          # 2 tiles per half
                    for half in range(2):
                        hsl = slice(half * HB, (half + 1) * HB)
                        hh = phD.tile([128, FCH, HB], HDT, tag="hh")
                        for piece in range(4):
                            pk = slice(piece * (FCH // 4), (piece + 1) * (FCH // 4))
                            nc.sync.dma_start(out=hh[:, pk, :], in_=hbuf[:, pk, hsl])
                        chs = []
                        for nn in range(NTH):
                            ch = pcD.tile([128, CH, N], F32R, tag="ch")
                            nc.sync.dma_start(
                                out=ch[:],
                                in_=cbuf[:, :, half * HB + nn * N: half * HB + (nn + 1) * N])
                            chs.append(ch)
                        for m in range(CH):
                            wm = pwm.tile([128, FCH, 128], HDT, tag="wm")
                            nc.sync.dma_start(out=wm[:], in_=wf2r[:, :, ts(m, 128)])
                            for nn in range(NTH):
                                acc = psD.tile([128, N], F32, tag="accD")
                                for k in range(FCH):
                                    nc.tensor.matmul(acc[:], wm[:, k, :],
                                                     hh[:, k, nn * N:(nn + 1) * N],
                                                     start=(k == 0), stop=(k == FCH - 1))
                                nc.vector.scalar_tensor_tensor(
                                    out=chs[nn][:, m, :], in0=acc[:],
                                    scalar=bf2sb[:, m:m + 1],
                                    in1=chs[nn][:, m, :], op0=OP.add, op1=OP.add)
                        for nn in range(NTH):
                            osl = slice(half * HB + nn * N, half * HB + (nn + 1) * N)
                            layer_norm(lnpoolsD, chs[nn], N, 2, chs[nn])
                            nc.sync.dma_start(out=otr[:, :, osl], in_=chs[nn][:])

    nc.finalize()
    return nc


def _build(repeats=1):
    from concourse import bacc

    nc = bacc.Bacc()
    return _emit_program(nc, repeats=repeats)


def _make_exec(nc, n_cores=NCORES):
    """Cached jitted SPMD executor, mirroring run_bass_via_pjrt's multi-core
    branch so repeated calls reuse the compiled NEFF."""
    import jax
    import concourse.mybir as mybir
    from concourse import bass2jax
    from jax.experimental.shard_map import shard_map
    from jax.sharding import Mesh, PartitionSpec

    bass2jax.install_neuronx_cc_hook()

    partition_name = nc.partition_id_tensor.name if nc.partition_id_tensor else None
    in_names, out_names, out_avals, zero_shapes = [], [], [], []
    for alloc in nc.m.functions[0].allocations:
        if not isinstance(alloc, mybir.MemoryLocationSet):
            continue
        name = alloc.memorylocations[0].name
        if alloc.kind == "ExternalInput":
            if name != partition_name:
                in_names.append(name)
        elif alloc.kind == "ExternalOutput":
            out_names.append(name)
            shape = tuple(alloc.tensor_shape)
            dtype = mybir.dt.np(alloc.dtype)
            out_avals.append(jax.core.ShapedArray(shape, dtype))
            zero_shapes.append((shape, dtype))
    n_params = len(in_names)
    n_outs = len(out_names)
    all_names = in_names + out_names
    if partition_name is not None:
        all_names = all_names + [partition_name]

    def _body(*args):
        operands = list(args)
        if partition_name is not None:
            operands.append(bass2jax.partition_id_tensor())
        outs = bass2jax._bass_exec_p.bind(
            *operands,
            out_avals=tuple(out_avals),
            in_names=tuple(all_names),
            out_names=tuple(out_names),
            lowering_input_output_aliases=(),
            sim_require_finite=True,
            sim_require_nnan=True,
            nc=nc,
        )
        return tuple(outs)

    devices = jax.devices()[:n_cores]
    mesh = Mesh(np.asarray(devices), ("core",))
    in_specs = (PartitionSpec("core"),) * (n_params + n_outs)
    out_specs = (PartitionSpec("core"),) * n_outs
    donate = tuple(range(n_params, n_params + n_outs))
    sharded = jax.jit(
        shard_map(_body, mesh=mesh, in_specs=in_specs, out_specs=out_specs,
                  check_rep=False),
        donate_argnums=donate, keep_unused=True)

    def run(in_maps):
        concat_in = [
            np.concatenate([np.asarray(in_maps[c][nm]) for c in range(n_cores)], axis=0)
            for nm in in_names
        ]
        concat_zeros = [
            np.zeros((n_cores * s[0],) + tuple(s[1:]), dt) for (s, dt) in zero_shapes
        ]
        out_arrs = sharded(*concat_in, *concat_zeros)
        out_arrs = [np.asarray(a) for a in out_arrs]
        return [
            {nm: out_arrs[i].reshape(n_cores, *out_avals[i].shape)[c]
             for i, nm in enumerate(out_names)}
            for c in range(n_cores)
        ]

    run.in_names = in_names
    run.out_names = out_names
    run.sharded = sharded
    run.n_cores = n_cores
    run.out_avals = out_avals
    run.zero_shapes = zero_shapes
    run.body = _body
    run.mesh = mesh
    run.in_specs = in_specs
    run.out_specs = out_specs
    run.nc = nc
    return run


def _pack_pp(v, ch):
    """bias vector [ch*128] -> per-partition [128, ch]."""
    return np.ascontiguousarray(v.reshape(ch, 128).T.astype(np.float32))


def prepare_in_maps(img_feat, txt_feat, w_in1, b_in1, w_out1, b_out1,
                    w_in2, b_in2, w_out2, b_out2,
                    g1, be1, g2, be2, g3, be3,
                    w_ffn1, b_ffn1, w_ffn2, b_ffn2):
    f32 = np.float32
    img = np.asarray(img_feat, f32)
    txt = np.asarray(txt_feat, f32)
    w_in1 = np.asarray(w_in1, f32); b_in1 = np.asarray(b_in1, f32)
    w_out1 = np.asarray(w_out1, f32); b_out1 = np.asarray(b_out1, f32)
    w_in2 = np.asarray(w_in2, f32); b_in2 = np.asarray(b_in2, f32)
    w_out2 = np.asarray(w_out2, f32); b_out2 = np.asarray(b_out2, f32)
    w_ffn1 = np.asarray(w_ffn1, f32); b_ffn1 = np.asarray(b_ffn1, f32)
    w_ffn2 = np.asarray(w_ffn2, f32); b_ffn2 = np.asarray(b_ffn2, f32)

    wv1 = w_in1[2 * E:]
    bv1 = b_in1[2 * E:]
    W1 = w_out1 @ wv1                      # att1 == txt @ W1.T + b1
    b1 = w_out1 @ bv1 + b_out1
    wv2 = w_in2[2 * E:]
    bv2 = b_in2[2 * E:]
    W2 = w_out2 @ wv2
    b2 = w_out2 @ bv2 + b_out2

    lnp = np.concatenate([
        _pack_pp(np.asarray(v, f32), CH)
        for v in (g1, be1, g2, be2, g3, be3)], axis=1)

    if _H_DT == "bf16":
        import ml_dtypes
        wf2t_host = np.ascontiguousarray(w_ffn2.T.astype(ml_dtypes.bfloat16))
    else:
        wf2t_host = np.ascontiguousarray(w_ffn2.T)
    shared = {
        "w1t": np.ascontiguousarray(W1.T),
        "w2t": np.ascontiguousarray(W2.T),
        "wf1t": np.ascontiguousarray(w_ffn1.T),
        "wf2t": wf2t_host,
        "bias1": _pack_pp(b1, CH),
        "bias2": _pack_pp(b2, CH),
        "bf1": _pack_pp(b_ffn1, FCH),
        "bf2": _pack_pp(b_ffn2, CH),
        "lnp": lnp,
        "ones_in": np.ones((128, 1), f32),
        "ones1_in": np.ones((1, 128), f32),
    }
    in_maps = []
    for c in range(NCORES):
        sh = slice(c * R, (c + 1) * R)
        m = dict(shared)
        m["xt"] = np.ascontiguousarray(txt[sh].T)
        m["it"] = np.ascontiguousarray(img[sh].T)
        in_maps.append(m)
    return in_maps


def get_runner():
    global _RUNNER
    if _RUNNER is None:
        nc = _build()
        _RUNNER = _make_exec(nc)
    return _RUNNER


def kernel(**inputs) -> np.ndarray:
    run = get_runner()
    in_maps = prepare_in_maps(**inputs)
    results = run(in_maps)
    out = np.empty((B, E), np.float32)
    for c in range(NCORES):
        out[c * R:(c + 1) * R] = results[c]["ot"].T
    return out
